# revision 43
# baseline (speedup 1.0000x reference)
"""Causal multi-head attention with relative position bias on 8 Trainium2
NeuronCores.

Problem (full shapes): x[2,2048,1024], rel_bias[16,2048,2048],
w_qkv[1024,3072], b_qkv[3072], w_out[1024,1024], b_out[1024].

Sharding: core = (batch, head-group): 2 batches x 4 head-groups of 4 heads.
Each core computes q/k/v projections for its 4 heads, causal attention with
rel-bias, and a partial output projection through its heads' rows of w_out.
Host sums the 4 partial outputs per batch (the tensor-parallel reduce) and
adds b_out.

Device kernel design notes:
- Scores are computed TRANSPOSED (scoresT[kj,qi] = k.q) so no on-chip
  transposes are needed anywhere: softmax reduction over keys becomes a
  matmul contraction, handled by appending a ones-column to V; the PV matmul
  directly produces the transposed attention output that the out-projection
  needs as its stationary operand.
- exp(score + bias) = exp(score) * exp(bias): host precomputes exp(rel_biasT)
  in bf16 with the causal mask baked in as exact zeros. ACT does a pure exp
  straight from PSUM; DVE multiplies two bf16 SBUF operands at 2x rate.
- The whole kernel is emitted as ONE software-pipelined instruction stream:
  the attention inner loop is ACT(exp)-limited, so the ACT-free matmul work
  (qk/v projections, out projection) is interleaved into the attention
  stream as "filler" units.  This keeps the PE continuously busy, which
  matters twice: engine idle time, and the PE p-state ramp (the PE only
  reaches 2.4 GHz after ~3us of continuous execution; gaps drop it to
  1.2 GHz).
- Scores MMs for the two heads of a pair use tile_position row-tiling
  ((0,0)/(64,0)) so the K=64 matmuls execute concurrently.
- Normalization: denominator rows (PSUM partition 64 of the PV accumulator)
  are copied to SBUF, partition-broadcast on the idle GpSimd engine, and
  inverted with the fast all-lane reciprocal_approx_fast - never a
  single-lane vector.reciprocal, and nothing on the TensorE critical path.
- PSUM budget (8 banks): 4 PV accumulators + 3 score tiles + 1 shared
  filler bank.
"""

import math
import sys
import types
from contextlib import ExitStack

import ml_dtypes
import numpy as np

B, S, D = 2, 2048, 1024
NH, HD = 16, 64
NCORES = 8
HPC = 4  # heads per core (2 pairs)

_BF16 = ml_dtypes.bfloat16

KC = D // 128   # 8 contraction chunks for the projections
NS4 = S // 512  # 4 s-superblocks
NSC = S // 128  # 16 s-chunks


def _install_ntff_hook():
    """concourse.bass_utils imports antenv.axon_hooks for NTFF tracing under
    axon; this container's antenv lacks that module. Provide it, backed by
    the ctypes hook from trn_agent_boot (if present)."""
    if "antenv.axon_hooks" in sys.modules:
        return
    try:
        import antenv
    except ImportError:
        return
    mod = types.ModuleType("antenv.axon_hooks")
    mod._hook = None
    mod.set_axon_ntff_profile_hook = lambda h: setattr(mod, "_hook", h)
    mod.get_axon_ntff_profile_hook = lambda: mod._hook
    sys.modules["antenv.axon_hooks"] = mod
    antenv.axon_hooks = mod
    try:
        from trn_agent_boot.trn_boot import _ntff_profile_via_ctypes

        h = _ntff_profile_via_ctypes("/opt/axon/libaxon_pjrt.so")
        if h is not None:
            mod._hook = h
    except Exception:
        pass


def _build_program(has_bqk: bool, has_bv: bool):
    import concourse.tile as tile
    from concourse import bacc, mybir

    bf = mybir.dt.bfloat16
    f32 = mybir.dt.float32
    EXP = mybir.ActivationFunctionType.Exp

    nc = bacc.Bacc("TRN2", target_bir_lowering=False, debug=False,
                   num_devices=NCORES)

    d = types.SimpleNamespace()
    d.xT = nc.dram_tensor("xT", [D, S], bf, kind="ExternalInput").ap()
    d.wqk = nc.dram_tensor("wqk", [D, 512], bf, kind="ExternalInput").ap()
    d.wv = nc.dram_tensor("wv", [D, 256], bf, kind="ExternalInput").ap()
    d.bqk = nc.dram_tensor("bqk", [4, 128], bf, kind="ExternalInput").ap()
    d.bv = nc.dram_tensor("bv", [1, 256], bf, kind="ExternalInput").ap()
    d.erb = nc.dram_tensor("erb", [HPC, S, S], bf, kind="ExternalInput").ap()
    d.wo = nc.dram_tensor("wo", [2, 128, D], bf, kind="ExternalInput").ap()
    d.out = nc.dram_tensor("out", [S, D], f32, kind="ExternalOutput").ap()

    st = types.SimpleNamespace()
    with tile.TileContext(nc) as tc:
        with ExitStack() as ctx:
            ep = ctx.enter_context
            # --- pools -------------------------------------------------
            xt_pool = ep(tc.tile_pool(name="xt", bufs=KC))
            wqk_pool = ep(tc.tile_pool(name="wqk", bufs=KC))
            wv_pool = ep(tc.tile_pool(name="wv", bufs=KC))
            wo_pool = ep(tc.tile_pool(name="wo", bufs=2))
            const_pool = ep(tc.tile_pool(name="consts", bufs=1))
            qkT_pool = ep(tc.tile_pool(name="qkT", bufs=16))
            v_pool = ep(tc.tile_pool(name="vsb", bufs=NSC))
            attnT_pool = ep(tc.tile_pool(name="attnT", bufs=8))
            esc_pool = ep(tc.tile_pool(name="esc", bufs=12))
            erb_pool = ep(tc.tile_pool(name="erb", bufs=18))
            pr_pool = ep(tc.tile_pool(name="prob", bufs=12))
            den_pool = ep(tc.tile_pool(name="den", bufs=3))
            bc_pool = ep(tc.tile_pool(name="bc", bufs=2))
            osb_pool = ep(tc.tile_pool(name="osb", bufs=4))
            dram_pool = ep(tc.tile_pool(name="dr", bufs=4, space="DRAM"))
            dacc_pool = ep(tc.tile_pool(name="dacc", bufs=6))
            # 8 PSUM banks: 3 PV accumulators (both heads col-tiled into
            # one bank) + 4 score tiles + 1 filler.
            fill_ps = ep(tc.tile_pool(name="fill_ps", bufs=1, space="PSUM"))
            sc_ps = ep(tc.tile_pool(name="sc_ps", bufs=4, space="PSUM"))
            pv_ps = ep(tc.tile_pool(name="pv_ps", bufs=3, space="PSUM"))

            # --- consts ------------------------------------------------
            st.ones_row = const_pool.tile([1, 512], bf)
            nc.gpsimd.memset(st.ones_row[:], 1.0)
            ones_f32 = const_pool.tile([128, 1], mybir.dt.float32,
                                       name="ones_f32", tag="ones_f32")
            nc.gpsimd.memset(ones_f32[:], 1.0)
            st.ones_col = const_pool.tile([128, 1], mybir.dt.float32r,
                                          name="ones_col", tag="ones_col")
            nc.vector.tensor_copy(st.ones_col[:], ones_f32[:])

            # --- loads (emission order = rough completion order) -------
            st.wqk_t, st.xt_t, st.wv_t = [], [], []
            for k in range(KC):
                w = wqk_pool.tile([128, 512], bf)
                nc.sync.dma_start(w[:], d.wqk[k * 128:(k + 1) * 128, :])
                st.wqk_t.append(w)
                xt = xt_pool.tile([128, S], bf)
                nc.sync.dma_start(xt[:], d.xT[k * 128:(k + 1) * 128, :])
                st.xt_t.append(xt)
            for k in range(KC):
                wv = wv_pool.tile([128, 256], bf)
                nc.sync.dma_start(wv[:], d.wv[k * 128:(k + 1) * 128, :])
                st.wv_t.append(wv)
            st.wo_t = []
            for p in range(2):
                w = wo_pool.tile([128, D], bf)
                nc.sync.dma_start(w[:], d.wo[p])
                st.wo_t.append(w)
            if has_bqk:
                st.bqk_sb = []
                for m in range(4):
                    t = const_pool.tile([1, 128], bf, name=f"bqk{m}",
                                        tag=f"bqk{m}")
                    nc.sync.dma_start(t[:], d.bqk[m:m + 1, :])
                    st.bqk_sb.append(t)
            if has_bv:
                st.bv_sb = const_pool.tile([1, 256], bf)
                nc.sync.dma_start(st.bv_sb[:], d.bv[:])

            # --- persistent result tiles -------------------------------
            # qkT and attnT are stored as per-512-column tiles so every
            # producer writes exactly one whole tile and consumers read
            # sub-ranges of one tile: whole-tile RAW edges, no partial
            # overlap tracking.
            st.qkT_t = {(m, s4): qkT_pool.tile([128, 512], bf, name="qkT",
                                               tag="qkT")
                        for m in range(4) for s4 in range(NS4)}
            st.v_t = [v_pool.tile([128, 256], bf, name="vsb", tag="vsb")
                      for _ in range(NSC)]
            st.attnT_t = {(p, g): attnT_pool.tile([128, 512], bf,
                                                  name="attnT", tag="attnT")
                          for p in range(2) for g in range(4)}

            # --- filler units ------------------------------------------
            def qk_unit(m, s4):
                """One qk-projection chain: qkT[m][:, s4*512:...]."""
                ps = fill_ps.tile([128, 512], f32, name="fps", tag="fps")
                for k in range(KC):
                    nc.tensor.matmul(
                        ps[:],
                        st.wqk_t[k][:, m * 128:(m + 1) * 128],
                        st.xt_t[k][:, s4 * 512:(s4 + 1) * 512],
                        start=(k == 0),
                        stop=(k == KC - 1 and not has_bqk),
                    )
                if has_bqk:
                    nc.tensor.matmul(
                        ps[:], st.bqk_sb[m][:], st.ones_row[:, :],
                        start=False, stop=True,
                    )
                nc.vector.tensor_copy(st.qkT_t[(m, s4)][:], ps[:])

            def v_unit(si):
                """One v-projection chain: v_t[si] (4x 64 v-cols + ones)."""
                ps = fill_ps.tile([128, 512], f32, name="fps", tag="fps")
                for k in range(KC):
                    nc.tensor.matmul(
                        ps[:, 0:256],
                        st.xt_t[k][:, si * 128:(si + 1) * 128],
                        st.wv_t[k][:],
                        start=(k == 0),
                        stop=(k == KC - 1 and not has_bv),
                    )
                if has_bv:
                    nc.tensor.matmul(
                        ps[:, 0:256], st.ones_row[0:1, 0:128], st.bv_sb[:],
                        start=False, stop=True,
                    )
                nc.vector.tensor_copy(st.v_t[si][:], ps[:, 0:256])

            def out_unit(si, e2, pool=None):
                """One out-projection tile: out[si*128:.., e2*512:..]."""
                if pool is None:
                    ps = fill_ps.tile([128, 512], f32, name="fps", tag="fps")
                else:
                    ps = pool.tile([128, 512], f32, name="sc", tag="sc")
                for p in range(2):
                    nc.tensor.matmul(
                        ps[:],
                        st.attnT_t[(p, si // 4)][:, (si % 4) * 128:
                                                 (si % 4 + 1) * 128],
                        st.wo_t[p][:, e2 * 512:(e2 + 1) * 512],
                        start=(p == 0), stop=(p == 1),
                    )
                osb = osb_pool.tile([128, 512], f32, name="osb", tag="osb")
                if e2 == 0:
                    nc.vector.tensor_copy(osb[:], ps[:])
                else:
                    nc.scalar.copy(osb[:], ps[:])
                nc.sync.dma_start(
                    d.out[si * 128:(si + 1) * 128,
                          e2 * 512:(e2 + 1) * 512],
                    osb[:])

            # --- one attention window (p = head pair, qi8 = q half) ----
            def window(p, qi8, fills):
                """fills: list of (min_kj_idx, emit_fn), FIFO order."""
                w0 = qi8 * 1024
                w1 = w0 + 1024
                nkj = w1 // 128

                def offs(kj):
                    return list(range(max(w0, (kj * 128) // 512 * 512),
                                      w1, 512))

                pv = [pv_ps.tile([128, 512], f32, name="pv", tag="pv")
                      for _ in range(2)]  # [q4]: h0 rows 0-63, h1 rows 64-127
                dacc = [[dacc_pool.tile([128, 512], mybir.dt.float32r,
                                        name=f"dacc{h}", tag=f"dacc{h}")
                         for q4 in range(2)] for h in range(2)]
                erb_t = {}
                pr_t = {}

                def emit_erb(kj):
                    for off in offs(kj):
                        for h in range(2):
                            hl = 2 * p + h
                            rb = erb_pool.tile([128, 512], bf, name="erb",
                                               tag="erb")
                            nc.sync.dma_start(
                                rb[:],
                                d.erb[hl, kj * 128:(kj + 1) * 128,
                                      off:off + 512])
                            erb_t[(kj, off, h)] = rb

                def emit_sem(kj, off):
                    kT = st.qkT_t[(2 * p + 1, kj // 4)]
                    kc = (kj % 4) * 128
                    qT = st.qkT_t[(2 * p, off // 512)]
                    sc = [sc_ps.tile([128, 512], f32, name="sc",
                                     tag="sc") for _ in range(2)]
                    for h in range(2):
                        rows = slice(64 * h, 64 * h + 64)
                        nc.tensor.matmul(
                            sc[h][:],
                            kT[rows, kc:kc + 128],
                            qT[rows, :],
                            start=True, stop=True,
                            tile_position=(64 * h, 0),
                        )
                    q4 = (off - w0) // 512
                    for h in range(2):
                        esc = esc_pool.tile([128, 512], bf, name="esc",
                                            tag="esc")
                        nc.scalar.activation(esc[:], sc[h][:], EXP)
                        pr = pr_pool.tile([128, 512], bf, name="prob",
                                          tag="prob")
                        nc.vector.tensor_mul(
                            pr[:], esc[:], erb_t.pop((kj, off, h))[:])
                        pr_t[(kj, off, h)] = pr
                        # softmax denominator partials accumulate in fp32
                        # on the otherwise idle GpSimd engine.
                        if kj == 0:
                            nc.gpsimd.tensor_copy(dacc[h][q4][:], pr[:])
                        else:
                            nc.gpsimd.tensor_add(
                                dacc[h][q4][:], dacc[h][q4][:], pr[:])

                def emit_norm(q4):
                    den = []
                    for h in range(2):
                        # reduce the fp32 partials across partitions with a
                        # single M=1 f32r matmul into a scratch PSUM row.
                        dp = sc_ps.tile([128, 512], f32, name="sc",
                                        tag="sc")
                        nc.tensor.matmul(
                            dp[0:1, :], st.ones_col[:], dacc[h][q4][:],
                            start=True, stop=True)
                        t = den_pool.tile([1, 512], f32, name=f"den{h}",
                                          tag=f"den{h}")
                        nc.vector.tensor_copy(t[:], dp[0:1, :])
                        den.append(t)
                    for h in range(2):
                        # partition-broadcast via a DRAM bounce (engines
                        # cannot broadcast an SBUF row across partitions).
                        dr = dram_pool.tile([1, 512], f32, name=f"dr{h}",
                                            tag=f"dr{h}")
                        nc.sync.dma_start(dr[:], den[h][:])
                        bcd = bc_pool.tile([64, 512], f32, name=f"bcd{h}",
                                           tag=f"bcd{h}")
                        nc.sync.dma_start(
                            bcd[:], dr[0:1, :].partition_broadcast(64))
                        bcr = bc_pool.tile([64, 512], f32, name=f"bc{h}",
                                           tag=f"bc{h}")
                        nc.vector.reciprocal_approx_fast(bcr[:], bcd[:])
                        nc.vector.tensor_mul(
                            st.attnT_t[(p, qi8 * 2 + q4)][64 * h:64 * h + 64,
                                                          :],
                            pv[q4][64 * h:64 * h + 64, :],
                            bcr[:])

                def emit_pv(kj, off):
                    q4 = (off - w0) // 512
                    last_kj = 8 * qi8 + 4 * q4 + 3
                    for h in range(2):
                        hl = 2 * p + h
                        nc.tensor.matmul(
                            pv[q4][64 * h:64 * h + 64, :],
                            st.v_t[kj][:, 64 * hl:64 * (hl + 1)],
                            pr_t.pop((kj, off, h))[:],
                            start=(kj == 0),
                            stop=(kj == last_kj),
                            tile_position=(0, 64 * h),
                        )
                    if kj == last_kj:
                        emit_norm(q4)

                units = [(kj, off) for kj in range(nkj)
                         for off in offs(kj)]
                n = len(units)
                PF = 4  # erb prefetch distance in units
                for kj in range(min(2, nkj)):
                    emit_erb(kj)
                emit_sem(*units[0])
                nf = len(fills)
                fi = 0
                ei = 2  # next kj to emit erb for
                for i in range(n):
                    kj = units[i][0]
                    if ei < nkj and units[min(i + PF, n - 1)][0] + 1 >= ei:
                        emit_erb(ei)
                        ei += 1
                    if i + 1 < n:
                        emit_sem(*units[i + 1])
                    target = (nf * (i + 1)) // n
                    while (fi < nf and fi < target
                           and fills[fi][0] <= kj):
                        fills[fi][1]()
                        fi += 1
                    emit_pv(*units[i])
                while fi < nf:
                    fills[fi][1]()
                    fi += 1

            # --- the schedule ------------------------------------------
            def F(fn, *a):
                return (0, (lambda: fn(*a)))

            # pre-phase: the minimum needed for window (p0, q0) kj0.
            qk_unit(0, 0)
            qk_unit(0, 1)
            qk_unit(1, 0)
            v_unit(0)

            window(0, 0, [
                F(v_unit, 1), F(v_unit, 2), F(qk_unit, 1, 1),
                F(v_unit, 3), F(qk_unit, 2, 0), F(v_unit, 4),
                F(qk_unit, 2, 1), F(v_unit, 5), F(qk_unit, 3, 0),
                F(v_unit, 6), F(v_unit, 7),
            ])
            window(1, 0, [
                F(qk_unit, 3, 1), F(qk_unit, 0, 2), F(v_unit, 8),
                F(qk_unit, 0, 3), F(v_unit, 9), F(v_unit, 10),
                F(v_unit, 11), F(v_unit, 12), F(v_unit, 13),
            ])
            w2_fills = [
                F(qk_unit, 1, 2), F(qk_unit, 2, 2), F(qk_unit, 1, 3),
                F(qk_unit, 2, 3), F(qk_unit, 3, 2), F(qk_unit, 3, 3),
                F(v_unit, 14), F(v_unit, 15),
            ]
            for si in range(0, 4):
                for e2 in range(2):
                    w2_fills.append((2, (lambda si=si, e2=e2:
                                         out_unit(si, e2))))
            window(0, 1, w2_fills)
            w3_fills = []
            for si in range(4, 8):
                for e2 in range(2):
                    w3_fills.append((0, (lambda si=si, e2=e2:
                                         out_unit(si, e2))))
            window(1, 1, w3_fills)
            # tail: alternate between the filler bank and the (now idle)
            # score banks so the units pipeline instead of serializing.
            for i, (si, e2) in enumerate(
                    [(si, e2) for si in range(8, NSC) for e2 in range(2)]):
                out_unit(si, e2, pool=None if i % 2 == 0 else sc_ps)

    nc.compile()
    return nc


_PROGRAM_CACHE = {}


def _get_program(has_bqk, has_bv):
    key = (has_bqk, has_bv)
    if key not in _PROGRAM_CACHE:
        _PROGRAM_CACHE[key] = _build_program(has_bqk, has_bv)
    return _PROGRAM_CACHE[key]


_last_results = None  # BassKernelResults of the most recent run (for test.py)


def kernel(x, rel_bias, w_qkv, b_qkv, w_out, b_out, *, trace=False):
    global _last_results
    _install_ntff_hook()
    from concourse.bass_utils import run_bass_kernel_spmd

    x = np.asarray(x, dtype=np.float32)
    rel_bias = np.asarray(rel_bias, dtype=np.float32)
    w_qkv = np.asarray(w_qkv, dtype=np.float32)
    b_qkv = np.asarray(b_qkv, dtype=np.float32)
    w_out = np.asarray(w_out, dtype=np.float32)
    b_out = np.asarray(b_out, dtype=np.float32)

    wq = w_qkv[:, 0:D]
    wk = w_qkv[:, D:2 * D]
    wv = w_qkv[:, 2 * D:3 * D]
    bq, bk, bv = b_qkv[0:D], b_qkv[D:2 * D], b_qkv[2 * D:3 * D]
    has_bqk = bool(np.any(bq)) or bool(np.any(bk))
    has_bv = bool(np.any(bv))

    nc = _get_program(has_bqk, has_bv)

    sc = 1.0 / math.sqrt(HD)  # folded into the q projection
    xT = [np.ascontiguousarray(x[b].T).astype(_BF16) for b in range(B)]
    tri = np.triu(np.ones((S, S), dtype=np.float32))  # [kj, qi]: qi >= kj

    in_maps = []
    for c in range(NCORES):
        b, hg = divmod(c, 4)
        hs = [4 * hg + i for i in range(HPC)]

        # wqk columns: [q_h0 | q_h1 | k_h0 | k_h1 | q_h2 | q_h3 | k_h2 | k_h3]
        cols = []
        bqk_rows = []
        for pair in range(2):
            h0, h1 = hs[2 * pair], hs[2 * pair + 1]
            cols += [wq[:, HD * h0:HD * (h0 + 1)] * sc,
                     wq[:, HD * h1:HD * (h1 + 1)] * sc]
            bqk_rows.append(np.concatenate(
                [bq[HD * h0:HD * (h0 + 1)], bq[HD * h1:HD * (h1 + 1)]]) * sc)
            cols += [wk[:, HD * h0:HD * (h0 + 1)],
                     wk[:, HD * h1:HD * (h1 + 1)]]
            bqk_rows.append(np.concatenate(
                [bk[HD * h0:HD * (h0 + 1)], bk[HD * h1:HD * (h1 + 1)]]))
        wqk_c = np.concatenate(cols, axis=1).astype(_BF16)
        bqk_c = np.stack(bqk_rows).astype(_BF16)

        wv_c = np.zeros((D, 256), dtype=np.float32)
        bv_c = np.zeros((1, 256), dtype=np.float32)
        for i, h in enumerate(hs):
            wv_c[:, 64 * i:64 * (i + 1)] = wv[:, HD * h:HD * (h + 1)]
            bv_c[0, 64 * i:64 * (i + 1)] = bv[HD * h:HD * (h + 1)]

        erb_c = np.empty((HPC, S, S), dtype=_BF16)
        for i, h in enumerate(hs):
            erb_c[i] = (np.exp(rel_bias[h].T) * tri).astype(_BF16)

        in_maps.append({
            "xT": xT[b],
            "wqk": wqk_c,
            "wv": wv_c.astype(_BF16),
            "bqk": bqk_c,
            "bv": bv_c.astype(_BF16),
            "erb": erb_c,
            "wo": np.ascontiguousarray(
                w_out[256 * hg:256 * (hg + 1)].reshape(2, 128, D)).astype(_BF16),
        })

    res = run_bass_kernel_spmd(nc, in_maps, list(range(NCORES)), trace=trace)
    _last_results = res

    out = np.zeros((B, S, D), dtype=np.float32)
    for c in range(NCORES):
        out[c // 4] += res.results[c]["out"]
    out += b_out
    return out


# revision 44
# speedup vs baseline: 1.3850x; 1.3850x over previous
"""Causal multi-head attention with relative position bias on 8 Trainium2
NeuronCores.

Problem (full shapes): x[2,2048,1024], rel_bias[16,2048,2048],
w_qkv[1024,3072], b_qkv[3072], w_out[1024,1024], b_out[1024].

Sharding: core = (batch, head-group): 2 batches x 4 head-groups of 4 heads.
Each core computes q/k/v projections for its 4 heads, causal attention with
rel-bias, and a partial output projection through its heads' rows of w_out.
Host sums the 4 partial outputs per batch (the tensor-parallel reduce) and
adds b_out.

Device kernel design notes:
- Scores are computed TRANSPOSED (scoresT[kj,qi] = k.q) so no on-chip
  transposes are needed anywhere: softmax reduction over keys becomes a
  matmul contraction, handled by appending a ones-column to V; the PV matmul
  directly produces the transposed attention output that the out-projection
  needs as its stationary operand.
- exp(score + bias) = exp(score) * exp(bias): host precomputes exp(rel_biasT)
  in bf16 with the causal mask baked in as exact zeros. ACT does a pure exp
  straight from PSUM; DVE multiplies two bf16 SBUF operands at 2x rate.
- The whole kernel is emitted as ONE software-pipelined instruction stream:
  the attention inner loop is ACT(exp)-limited, so the ACT-free matmul work
  (qk/v projections, out projection) is interleaved into the attention
  stream as "filler" units.  This keeps the PE continuously busy, which
  matters twice: engine idle time, and the PE p-state ramp (the PE only
  reaches 2.4 GHz after ~3us of continuous execution; gaps drop it to
  1.2 GHz).
- Scores MMs for the two heads of a pair use tile_position row-tiling
  ((0,0)/(64,0)) so the K=64 matmuls execute concurrently.
- Normalization: denominator rows (PSUM partition 64 of the PV accumulator)
  are copied to SBUF, partition-broadcast on the idle GpSimd engine, and
  inverted with the fast all-lane reciprocal_approx_fast - never a
  single-lane vector.reciprocal, and nothing on the TensorE critical path.
- PSUM budget (8 banks): 4 PV accumulators + 3 score tiles + 1 shared
  filler bank.
"""

import math
import sys
import types
from contextlib import ExitStack

import ml_dtypes
import numpy as np

B, S, D = 2, 2048, 1024
NH, HD = 16, 64
NCORES = 8
HPC = 4  # heads per core (2 pairs)

_BF16 = ml_dtypes.bfloat16

KC = D // 128   # 8 contraction chunks for the projections
NS4 = S // 512  # 4 s-superblocks
NSC = S // 128  # 16 s-chunks


def _install_ntff_hook():
    """concourse.bass_utils imports antenv.axon_hooks for NTFF tracing under
    axon; this container's antenv lacks that module. Provide it, backed by
    the ctypes hook from trn_agent_boot (if present)."""
    if "antenv.axon_hooks" in sys.modules:
        return
    try:
        import antenv
    except ImportError:
        return
    mod = types.ModuleType("antenv.axon_hooks")
    mod._hook = None
    mod.set_axon_ntff_profile_hook = lambda h: setattr(mod, "_hook", h)
    mod.get_axon_ntff_profile_hook = lambda: mod._hook
    sys.modules["antenv.axon_hooks"] = mod
    antenv.axon_hooks = mod
    try:
        from trn_agent_boot.trn_boot import _ntff_profile_via_ctypes

        h = _ntff_profile_via_ctypes("/opt/axon/libaxon_pjrt.so")
        if h is not None:
            mod._hook = h
    except Exception:
        pass


def _build_program(has_bqk: bool, has_bv: bool):
    import concourse.tile as tile
    from concourse import bacc, mybir

    bf = mybir.dt.bfloat16
    f32 = mybir.dt.float32
    EXP = mybir.ActivationFunctionType.Exp

    nc = bacc.Bacc("TRN2", target_bir_lowering=False, debug=False,
                   num_devices=NCORES)

    d = types.SimpleNamespace()
    d.xT = nc.dram_tensor("xT", [D, S], bf, kind="ExternalInput").ap()
    d.wqk = nc.dram_tensor("wqk", [D, 512], bf, kind="ExternalInput").ap()
    d.wv = nc.dram_tensor("wv", [D, 260], bf, kind="ExternalInput").ap()
    d.bqk = nc.dram_tensor("bqk", [4, 128], bf, kind="ExternalInput").ap()
    d.bv = nc.dram_tensor("bv", [1, 260], bf, kind="ExternalInput").ap()
    d.erb = nc.dram_tensor("erb", [HPC, S, S], bf, kind="ExternalInput").ap()
    d.wo = nc.dram_tensor("wo", [2, 128, D], bf, kind="ExternalInput").ap()
    d.out = nc.dram_tensor("out", [S, D], f32, kind="ExternalOutput").ap()

    st = types.SimpleNamespace()
    with tile.TileContext(nc) as tc:
        with ExitStack() as ctx:
            ep = ctx.enter_context
            # --- pools -------------------------------------------------
            xt_pool = ep(tc.tile_pool(name="xt", bufs=KC))
            wqk_pool = ep(tc.tile_pool(name="wqk", bufs=KC))
            wv_pool = ep(tc.tile_pool(name="wv", bufs=KC))
            wo_pool = ep(tc.tile_pool(name="wo", bufs=2))
            const_pool = ep(tc.tile_pool(name="consts", bufs=1))
            qkT_pool = ep(tc.tile_pool(name="qkT", bufs=16))
            v_pool = ep(tc.tile_pool(name="vsb", bufs=NSC))
            attnT_pool = ep(tc.tile_pool(name="attnT", bufs=8))
            esc_pool = ep(tc.tile_pool(name="esc", bufs=12))
            erb_pool = ep(tc.tile_pool(name="erb", bufs=18))
            pr_pool = ep(tc.tile_pool(name="prob", bufs=12))
            den_pool = ep(tc.tile_pool(name="den", bufs=3))
            bc_pool = ep(tc.tile_pool(name="bc", bufs=2))
            osb_pool = ep(tc.tile_pool(name="osb", bufs=4))
            dram_pool = ep(tc.tile_pool(name="dr", bufs=4, space="DRAM"))
            # 8 PSUM banks: 4 PV accumulators + 3 score tiles + 1 filler.
            fill_ps = ep(tc.tile_pool(name="fill_ps", bufs=1, space="PSUM"))
            sc_ps = ep(tc.tile_pool(name="sc_ps", bufs=3, space="PSUM"))
            pv_ps = ep(tc.tile_pool(name="pv_ps", bufs=4, space="PSUM"))

            # --- consts ------------------------------------------------
            st.ones_row = const_pool.tile([1, 512], bf)
            nc.gpsimd.memset(st.ones_row[:], 1.0)
            ones_f32 = const_pool.tile([128, 1], mybir.dt.float32,
                                       name="ones_f32", tag="ones_f32")
            nc.gpsimd.memset(ones_f32[:], 1.0)
            st.ones_col = const_pool.tile([128, 1], mybir.dt.float32r,
                                          name="ones_col", tag="ones_col")
            nc.vector.tensor_copy(st.ones_col[:], ones_f32[:])

            # --- loads (emission order = rough completion order) -------
            st.wqk_t, st.xt_t, st.wv_t = [], [], []
            for k in range(KC):
                w = wqk_pool.tile([128, 512], bf)
                nc.sync.dma_start(w[:], d.wqk[k * 128:(k + 1) * 128, :])
                st.wqk_t.append(w)
                xt = xt_pool.tile([128, S], bf)
                nc.sync.dma_start(xt[:], d.xT[k * 128:(k + 1) * 128, :])
                st.xt_t.append(xt)
            for k in range(KC):
                wv = wv_pool.tile([128, 260], bf)
                nc.sync.dma_start(wv[:], d.wv[k * 128:(k + 1) * 128, :])
                st.wv_t.append(wv)
            st.wo_t = []
            for p in range(2):
                w = wo_pool.tile([128, D], bf)
                nc.sync.dma_start(w[:], d.wo[p])
                st.wo_t.append(w)
            if has_bqk:
                st.bqk_sb = []
                for m in range(4):
                    t = const_pool.tile([1, 128], bf, name=f"bqk{m}",
                                        tag=f"bqk{m}")
                    nc.sync.dma_start(t[:], d.bqk[m:m + 1, :])
                    st.bqk_sb.append(t)
            if has_bv:
                st.bv_sb = const_pool.tile([1, 260], bf)
                nc.sync.dma_start(st.bv_sb[:], d.bv[:])

            # --- persistent result tiles -------------------------------
            # qkT and attnT are stored as per-512-column tiles so every
            # producer writes exactly one whole tile and consumers read
            # sub-ranges of one tile: whole-tile RAW edges, no partial
            # overlap tracking.
            st.qkT_t = {(m, s4): qkT_pool.tile([128, 512], bf, name="qkT",
                                               tag="qkT")
                        for m in range(4) for s4 in range(NS4)}
            st.v_t = [v_pool.tile([128, 260], bf, name="vsb", tag="vsb")
                      for _ in range(NSC)]
            st.attnT_t = {(p, g): attnT_pool.tile([128, 512], bf,
                                                  name="attnT", tag="attnT")
                          for p in range(2) for g in range(4)}

            # --- filler units ------------------------------------------
            def qk_unit(m, s4):
                """One qk-projection chain: qkT[m][:, s4*512:...]."""
                ps = fill_ps.tile([128, 512], f32, name="fps", tag="fps")
                for k in range(KC):
                    nc.tensor.matmul(
                        ps[:],
                        st.wqk_t[k][:, m * 128:(m + 1) * 128],
                        st.xt_t[k][:, s4 * 512:(s4 + 1) * 512],
                        start=(k == 0),
                        stop=(k == KC - 1 and not has_bqk),
                    )
                if has_bqk:
                    nc.tensor.matmul(
                        ps[:], st.bqk_sb[m][:], st.ones_row[:, :],
                        start=False, stop=True,
                    )
                nc.vector.tensor_copy(st.qkT_t[(m, s4)][:], ps[:])

            def v_unit(si):
                """One v-projection chain: v_t[si] (4x 64 v-cols + ones)."""
                ps = fill_ps.tile([128, 512], f32, name="fps", tag="fps")
                for k in range(KC):
                    nc.tensor.matmul(
                        ps[:, 0:260],
                        st.xt_t[k][:, si * 128:(si + 1) * 128],
                        st.wv_t[k][:],
                        start=(k == 0),
                        stop=(k == KC - 1 and not has_bv),
                    )
                if has_bv:
                    nc.tensor.matmul(
                        ps[:, 0:260], st.ones_row[0:1, 0:128], st.bv_sb[:],
                        start=False, stop=True,
                    )
                nc.vector.tensor_copy(st.v_t[si][:], ps[:, 0:260])
                for h in range(HPC):
                    nc.gpsimd.memset(
                        st.v_t[si][:, 65 * h + 64:65 * h + 65], 1.0)

            def out_unit(si, e2, pool=None):
                """One out-projection tile: out[si*128:.., e2*512:..]."""
                if pool is None:
                    ps = fill_ps.tile([128, 512], f32, name="fps", tag="fps")
                else:
                    ps = pool.tile([128, 512], f32, name="sc", tag="sc")
                for p in range(2):
                    nc.tensor.matmul(
                        ps[:],
                        st.attnT_t[(p, si // 4)][:, (si % 4) * 128:
                                                 (si % 4 + 1) * 128],
                        st.wo_t[p][:, e2 * 512:(e2 + 1) * 512],
                        start=(p == 0), stop=(p == 1),
                    )
                osb = osb_pool.tile([128, 512], f32, name="osb", tag="osb")
                if e2 == 0:
                    nc.vector.tensor_copy(osb[:], ps[:])
                else:
                    nc.scalar.copy(osb[:], ps[:])
                nc.sync.dma_start(
                    d.out[si * 128:(si + 1) * 128,
                          e2 * 512:(e2 + 1) * 512],
                    osb[:])

            # --- one attention window (p = head pair, qi8 = q half) ----
            def window(p, qi8, fills):
                """fills: list of (min_kj_idx, emit_fn), FIFO order."""
                w0 = qi8 * 1024
                w1 = w0 + 1024
                nkj = w1 // 128

                def offs(kj):
                    return list(range(max(w0, (kj * 128) // 512 * 512),
                                      w1, 512))

                pv = [[pv_ps.tile([65, 512], f32, name="pv", tag="pv")
                       for _ in range(2)] for _ in range(2)]
                erb_t = {}
                pr_t = {}

                def emit_erb(kj):
                    for off in offs(kj):
                        for h in range(2):
                            hl = 2 * p + h
                            rb = erb_pool.tile([128, 512], bf, name="erb",
                                               tag="erb")
                            nc.sync.dma_start(
                                rb[:],
                                d.erb[hl, kj * 128:(kj + 1) * 128,
                                      off:off + 512])
                            erb_t[(kj, off, h)] = rb

                def emit_sem(kj, off):
                    kT = st.qkT_t[(2 * p + 1, kj // 4)]
                    kc = (kj % 4) * 128
                    qT = st.qkT_t[(2 * p, off // 512)]
                    sc = [sc_ps.tile([128, 512], f32, name="sc",
                                     tag="sc") for _ in range(2)]
                    for h in range(2):
                        rows = slice(64 * h, 64 * h + 64)
                        nc.tensor.matmul(
                            sc[h][:],
                            kT[rows, kc:kc + 128],
                            qT[rows, :],
                            start=True, stop=True,
                            tile_position=(64 * h, 0),
                        )
                    for h in range(2):
                        esc = esc_pool.tile([128, 512], bf, name="esc",
                                            tag="esc")
                        nc.scalar.activation(esc[:], sc[h][:], EXP)
                        pr = pr_pool.tile([128, 512], bf, name="prob",
                                          tag="prob")
                        nc.vector.tensor_mul(
                            pr[:], esc[:], erb_t.pop((kj, off, h))[:])
                        pr_t[(kj, off, h)] = pr

                def emit_norm(q4):
                    den = []
                    for h in range(2):
                        t = den_pool.tile([1, 512], f32, name=f"den{h}",
                                          tag=f"den{h}")
                        nc.vector.tensor_copy(t[:], pv[h][q4][64:65, :])
                        den.append(t)
                    for h in range(2):
                        # partition-broadcast via a DRAM bounce (engines
                        # cannot broadcast an SBUF row across partitions).
                        dr = dram_pool.tile([1, 512], f32, name=f"dr{h}",
                                            tag=f"dr{h}")
                        nc.sync.dma_start(dr[:], den[h][:])
                        bcd = bc_pool.tile([64, 512], f32, name=f"bcd{h}",
                                           tag=f"bcd{h}")
                        nc.sync.dma_start(
                            bcd[:], dr[0:1, :].partition_broadcast(64))
                        bcr = bc_pool.tile([64, 512], f32, name=f"bc{h}",
                                           tag=f"bc{h}")
                        nc.vector.reciprocal_approx_fast(bcr[:], bcd[:])
                        nc.vector.tensor_mul(
                            st.attnT_t[(p, qi8 * 2 + q4)][64 * h:64 * h + 64,
                                                          :],
                            pv[h][q4][0:64, :],
                            bcr[:])

                def emit_pv(kj, off):
                    q4 = (off - w0) // 512
                    last_kj = 8 * qi8 + 4 * q4 + 3
                    for h in range(2):
                        hl = 2 * p + h
                        nc.tensor.matmul(
                            pv[h][q4][:],
                            st.v_t[kj][:, 65 * hl:65 * hl + 65],
                            pr_t.pop((kj, off, h))[:],
                            start=(kj == 0),
                            stop=(kj == last_kj),
                        )
                    if kj == last_kj:
                        emit_norm(q4)

                units = [(kj, off) for kj in range(nkj)
                         for off in offs(kj)]
                n = len(units)
                PF = 4  # erb prefetch distance in units
                for kj in range(min(2, nkj)):
                    emit_erb(kj)
                emit_sem(*units[0])
                nf = len(fills)
                fi = 0
                ei = 2  # next kj to emit erb for
                for i in range(n):
                    kj = units[i][0]
                    if ei < nkj and units[min(i + PF, n - 1)][0] + 1 >= ei:
                        emit_erb(ei)
                        ei += 1
                    if i + 1 < n:
                        emit_sem(*units[i + 1])
                    target = (nf * (i + 1)) // n
                    while (fi < nf and fi < target
                           and fills[fi][0] <= kj):
                        fills[fi][1]()
                        fi += 1
                    emit_pv(*units[i])
                while fi < nf:
                    fills[fi][1]()
                    fi += 1

            # --- the schedule ------------------------------------------
            def F(fn, *a):
                return (0, (lambda: fn(*a)))

            # pre-phase: the minimum needed for window (p0, q0) kj0.
            qk_unit(0, 0)
            qk_unit(0, 1)
            qk_unit(1, 0)
            v_unit(0)

            window(0, 0, [
                F(v_unit, 1), F(v_unit, 2), F(qk_unit, 1, 1),
                F(v_unit, 3), F(qk_unit, 2, 0), F(v_unit, 4),
                F(qk_unit, 2, 1), F(v_unit, 5), F(qk_unit, 3, 0),
                F(v_unit, 6), F(v_unit, 7),
            ])
            window(1, 0, [
                F(qk_unit, 3, 1), F(qk_unit, 0, 2), F(v_unit, 8),
                F(qk_unit, 0, 3), F(v_unit, 9), F(v_unit, 10),
                F(v_unit, 11), F(v_unit, 12), F(v_unit, 13),
            ])
            w2_fills = [
                F(qk_unit, 1, 2), F(qk_unit, 2, 2), F(qk_unit, 1, 3),
                F(qk_unit, 2, 3), F(qk_unit, 3, 2), F(qk_unit, 3, 3),
                F(v_unit, 14), F(v_unit, 15),
            ]
            for si in range(0, 4):
                for e2 in range(2):
                    w2_fills.append((2, (lambda si=si, e2=e2:
                                         out_unit(si, e2))))
            window(0, 1, w2_fills)
            w3_fills = []
            for si in range(4, 8):
                for e2 in range(2):
                    w3_fills.append((0, (lambda si=si, e2=e2:
                                         out_unit(si, e2))))
            window(1, 1, w3_fills)
            # tail: alternate between the filler bank and the (now idle)
            # score banks so the units pipeline instead of serializing.
            for i, (si, e2) in enumerate(
                    [(si, e2) for si in range(8, NSC) for e2 in range(2)]):
                out_unit(si, e2, pool=None if i % 2 == 0 else sc_ps)

    nc.compile()
    return nc


_PROGRAM_CACHE = {}


def _get_program(has_bqk, has_bv):
    key = (has_bqk, has_bv)
    if key not in _PROGRAM_CACHE:
        _PROGRAM_CACHE[key] = _build_program(has_bqk, has_bv)
    return _PROGRAM_CACHE[key]


_last_results = None  # BassKernelResults of the most recent run (for test.py)


def kernel(x, rel_bias, w_qkv, b_qkv, w_out, b_out, *, trace=False):
    global _last_results
    _install_ntff_hook()
    from concourse.bass_utils import run_bass_kernel_spmd

    x = np.asarray(x, dtype=np.float32)
    rel_bias = np.asarray(rel_bias, dtype=np.float32)
    w_qkv = np.asarray(w_qkv, dtype=np.float32)
    b_qkv = np.asarray(b_qkv, dtype=np.float32)
    w_out = np.asarray(w_out, dtype=np.float32)
    b_out = np.asarray(b_out, dtype=np.float32)

    wq = w_qkv[:, 0:D]
    wk = w_qkv[:, D:2 * D]
    wv = w_qkv[:, 2 * D:3 * D]
    bq, bk, bv = b_qkv[0:D], b_qkv[D:2 * D], b_qkv[2 * D:3 * D]
    has_bqk = bool(np.any(bq)) or bool(np.any(bk))
    has_bv = bool(np.any(bv))

    nc = _get_program(has_bqk, has_bv)

    sc = 1.0 / math.sqrt(HD)  # folded into the q projection
    xT = [np.ascontiguousarray(x[b].T).astype(_BF16) for b in range(B)]
    tri = np.triu(np.ones((S, S), dtype=np.float32))  # [kj, qi]: qi >= kj

    in_maps = []
    for c in range(NCORES):
        b, hg = divmod(c, 4)
        hs = [4 * hg + i for i in range(HPC)]

        # wqk columns: [q_h0 | q_h1 | k_h0 | k_h1 | q_h2 | q_h3 | k_h2 | k_h3]
        cols = []
        bqk_rows = []
        for pair in range(2):
            h0, h1 = hs[2 * pair], hs[2 * pair + 1]
            cols += [wq[:, HD * h0:HD * (h0 + 1)] * sc,
                     wq[:, HD * h1:HD * (h1 + 1)] * sc]
            bqk_rows.append(np.concatenate(
                [bq[HD * h0:HD * (h0 + 1)], bq[HD * h1:HD * (h1 + 1)]]) * sc)
            cols += [wk[:, HD * h0:HD * (h0 + 1)],
                     wk[:, HD * h1:HD * (h1 + 1)]]
            bqk_rows.append(np.concatenate(
                [bk[HD * h0:HD * (h0 + 1)], bk[HD * h1:HD * (h1 + 1)]]))
        wqk_c = np.concatenate(cols, axis=1).astype(_BF16)
        bqk_c = np.stack(bqk_rows).astype(_BF16)

        wv_c = np.zeros((D, 260), dtype=np.float32)
        bv_c = np.zeros((1, 260), dtype=np.float32)
        for i, h in enumerate(hs):
            wv_c[:, 65 * i:65 * i + 64] = wv[:, HD * h:HD * (h + 1)]
            bv_c[0, 65 * i:65 * i + 64] = bv[HD * h:HD * (h + 1)]

        erb_c = np.empty((HPC, S, S), dtype=_BF16)
        for i, h in enumerate(hs):
            erb_c[i] = (np.exp(rel_bias[h].T) * tri).astype(_BF16)

        in_maps.append({
            "xT": xT[b],
            "wqk": wqk_c,
            "wv": wv_c.astype(_BF16),
            "bqk": bqk_c,
            "bv": bv_c.astype(_BF16),
            "erb": erb_c,
            "wo": np.ascontiguousarray(
                w_out[256 * hg:256 * (hg + 1)].reshape(2, 128, D)).astype(_BF16),
        })

    res = run_bass_kernel_spmd(nc, in_maps, list(range(NCORES)), trace=trace)
    _last_results = res

    out = np.zeros((B, S, D), dtype=np.float32)
    for c in range(NCORES):
        out[c // 4] += res.results[c]["out"]
    out += b_out
    return out
